# revision 11
# baseline (speedup 1.0000x reference)
"""CropAndResize (TF-style, crop 14x14) on 8 Trainium2 NeuronCores.

Strategy (data-parallel over ROIs, ring-balanced across cores):
  - Host: build per-image patch tensor img2[y*W+x] = the 2x2 corner patch
    (rows y,y+1c cols x,x+1c, edge-clamped) x 256 channels, bf16. One
    gather descriptor then fetches all 4 bilinear corners (2KB) for one
    output pixel.
  - Core k holds the patch tensors of images k and k+1 (mod 8); each box
    of image j runs on core j or j-1, chosen by a ring-balancing pass so
    every core gets the same box count (int16 gather indices span two
    images: 2*15200 < 32768).
  - Host: fold the bilinear lerp + validity mask into 4 corner weights
    per output pixel: out = w00*tl + w01*tr + w10*bl + w11*br.
  - Device: per chunk of 8 boxes, SWDGE dma_gather (round-robin over 4
    queues) fetches the patches (pixel on partitions, 4x256ch on the
    free dim); ACT does the first weighted term, DVE accumulates the
    other three via scalar_tensor_tensor with per-partition weights;
    result streams back to DRAM partition-major in bf16.
  - Host: scatter per-core outputs back to box order, upcast to f32.
"""

import ml_dtypes
import numpy as np

import concourse.bacc as bacc
import concourse.bass as bass
import concourse.tile as tile
from concourse import mybir, library_config, bass_utils

H, W, C = 100, 152, 256
CROP = 14
PX = CROP * CROP          # 196 pixels per box
P = 128                   # SBUF partitions
NCORES = 8
CH = 16                   # boxes per chunk
QPAD = ((CH * PX + P - 1) // P) * P   # padded pixels per chunk (1664)
S = QPAD // P             # output slots per chunk (13)
NPIX = H * W              # 15200 patch slots per image
EL = 4 * C                # gathered elements per pixel (4 corners x 256ch)
NQ = 4                    # SWDGE queues

BF16 = mybir.dt.bfloat16
F32 = mybir.dt.float32
I16 = mybir.dt.int16
MULT = mybir.AluOpType.mult
ADD = mybir.AluOpType.add

_cache = {}
LAST_EXEC_NS = None
LAST_RESULT = None


def _grid_params(boxes):
    """f32 mirror of the reference sampling-grid math."""
    f = np.float32
    y1, x1, y2, x2 = boxes[:, 0], boxes[:, 1], boxes[:, 2], boxes[:, 3]
    h_scale = (y2 - y1) * f(H - 1) / f(CROP - 1)
    w_scale = (x2 - x1) * f(W - 1) / f(CROP - 1)
    ar = np.arange(CROP, dtype=np.float32)
    in_y = y1[:, None] * f(H - 1) + ar[None, :] * h_scale[:, None]
    in_x = x1[:, None] * f(W - 1) + ar[None, :] * w_scale[:, None]
    valid_y = (in_y >= 0) & (in_y <= H - 1)
    valid_x = (in_x >= 0) & (in_x <= W - 1)
    top = np.floor(in_y)
    left = np.floor(in_x)
    y_lerp = (in_y - top).astype(np.float32)
    x_lerp = (in_x - left).astype(np.float32)
    ti = np.clip(top, 0, H - 1).astype(np.int32)
    li = np.clip(left, 0, W - 1).astype(np.int32)
    return ti, li, y_lerp, x_lerp, valid_y, valid_x


def _ring_balance(counts):
    """Split image j's boxes between cores j-1 and j so loads equalize.

    Returns (L, z) with z[j] = how many of image j's boxes go to core
    (j-1)%NCORES; max per-core load is L.
    """
    total = sum(counts)
    for L in range((total + NCORES - 1) // NCORES, max(counts) + 1):
        for t in range(counts[0] + 1):
            z = [t]
            ok = True
            for k in range(NCORES):
                znext = min(counts[(k + 1) % NCORES], L - (counts[k] - z[k]))
                if znext < 0:
                    ok = False
                    break
                z.append(znext)
            if ok and z[NCORES] >= z[0]:
                return L, z[:NCORES]
    return max(counts), [0] * NCORES


def _build_core_inputs(boxes_k, img_off):
    """Per-core gather indices + folded corner weights for m_pad boxes.

    img_off[b] is the patch-index offset (0 or NPIX) selecting which of
    the core's two resident images box b samples from.
    """
    m_pad = boxes_k.shape[0]
    assert m_pad % CH == 0
    nch = m_pad // CH
    ti, li, yl, xl, vy, vx = _grid_params(boxes_k)

    # per (box, i, j) flattened to q within each chunk
    patch = (ti[:, :, None] * W + li[:, None, :]).reshape(m_pad, PX)
    patch = patch + img_off[:, None]
    yl_q = np.broadcast_to(yl[:, :, None], (m_pad, CROP, CROP)).reshape(m_pad, PX)
    xl_q = np.broadcast_to(xl[:, None, :], (m_pad, CROP, CROP)).reshape(m_pad, PX)
    vm_q = (vy[:, :, None] & vx[:, None, :]).reshape(m_pad, PX).astype(np.float32)
    w00 = (1 - yl_q) * (1 - xl_q) * vm_q
    w01 = (1 - yl_q) * xl_q * vm_q
    w10 = yl_q * (1 - xl_q) * vm_q
    w11 = yl_q * xl_q * vm_q
    wq_all = np.stack([w00, w01, w10, w11], 1)  # [m_pad, 4, PX]

    idx_all = np.zeros((nch, QPAD), np.int16)
    w_all = np.zeros((nch, P, S * 4), np.float32)
    for ch in range(nch):
        sl = slice(ch * CH, (ch + 1) * CH)
        t = patch[sl].reshape(-1)
        idx_all[ch, : t.size] = t
        wq = np.zeros((4, QPAD), np.float32)
        wq[:, : t.size] = wq_all[sl].transpose(1, 0, 2).reshape(4, -1)
        # slot s, partition p <- q = s*128+p ; layout [P, S*4] = [p, s*4+c]
        w_all[ch] = (wq.reshape(4, S, P).transpose(2, 1, 0)
                     .reshape(P, S * 4))
    # wrapped idx layout: [16, QPAD//16] idx k at (k%16, k//16), tiled to 128
    wrapped = idx_all.reshape(nch, QPAD // 16, 16).transpose(0, 2, 1)
    idx_wrapped = np.tile(wrapped, (1, NCORES, 1))  # [nch, 128, QPAD//16]
    return idx_wrapped, w_all


def _build_patches(image_t16):
    """[H, W, C] fp16 -> [NPIX, 4*C] fp16 edge-clamped 2x2 corner patches."""
    imgp = np.empty((H + 1, W + 1, C), ml_dtypes.bfloat16)
    imgp[:H, :W] = image_t16
    imgp[H, :W] = image_t16[H - 1]
    imgp[:, W] = imgp[:, W - 1]
    img2 = np.empty((H, W, 4, C), ml_dtypes.bfloat16)
    img2[:, :, 0] = imgp[:H, :W]
    img2[:, :, 1] = imgp[:H, 1:]
    img2[:, :, 2] = imgp[1:, :W]
    img2[:, :, 3] = imgp[1:, 1:]
    return img2.reshape(NPIX * EL)


def _build_program(nch):
    nc = bacc.Bacc("TRN2", target_bir_lowering=False, debug=False,
                   num_devices=NCORES, num_swdge_queues=NQ,
                   dynamic_dma_scratch_size=32768)
    img = nc.dram_tensor("img", [2 * NPIX * EL], BF16, kind="ExternalInput")
    idx = nc.dram_tensor("idx", [nch, P, QPAD // 16], I16, kind="ExternalInput")
    wts = nc.dram_tensor("wts", [nch, P, S * 4], F32, kind="ExternalInput")
    out = nc.dram_tensor("out", [nch * P * S * C], BF16, kind="ExternalOutput")

    gather_src = bass.AP(img, 0, [(EL, 2 * NPIX), (1, EL)])

    with tile.TileContext(nc) as tc:
        with (
            tc.tile_pool(name="gat", bufs=3) as gat_pool,
            tc.tile_pool(name="osb", bufs=2) as out_pool,
            tc.tile_pool(name="meta", bufs=8) as meta_pool,
            tc.tile_pool(name="tmp", bufs=4) as tmp_pool,
        ):
            nc.gpsimd.load_library(library_config.mlp)
            qn = 0
            tiles = {}

            def emit_gather(ch):
                nonlocal qn
                idx_t = meta_pool.tile([P, QPAD // 16], I16, tag="idx")
                nc.sync.dma_start(idx_t[:], idx[ch])
                w_t = meta_pool.tile([P, S * 4], F32, tag="wts")
                nc.sync.dma_start(w_t[:], wts[ch])
                g = gat_pool.tile([P, S, EL], BF16, tag="g")
                GU = 512
                for j0 in range(0, QPAD, GU):
                    nj = min(GU, QPAD - j0)
                    nc.gpsimd.dma_gather(
                        g[:, j0 // P: (j0 + nj) // P, :], gather_src,
                        idx_t[:, j0 // 16: (j0 + nj) // 16], nj, nj,
                        EL, queue_num=qn)
                    qn = (qn + 1) % NQ
                tiles[ch] = (g, w_t)

            def emit_compute(ch):
                g, w_t = tiles.pop(ch)
                o = out_pool.tile([P, S, C], BF16, tag="o")
                for s in range(S):
                    t0 = g[:, s, 0:C]
                    t1 = g[:, s, C:2 * C]
                    b0 = g[:, s, 2 * C:3 * C]
                    b1 = g[:, s, 3 * C:4 * C]
                    w0 = w_t[:, s * 4 + 0: s * 4 + 1]
                    w1 = w_t[:, s * 4 + 1: s * 4 + 2]
                    w2 = w_t[:, s * 4 + 2: s * 4 + 3]
                    w3 = w_t[:, s * 4 + 3: s * 4 + 4]

                    a1 = tmp_pool.tile([P, C], BF16, tag="a1")
                    nc.scalar.mul(a1[:], t0, w0)
                    a2 = tmp_pool.tile([P, C], BF16, tag="a2")
                    nc.vector.scalar_tensor_tensor(a2[:], t1, w1, a1[:],
                                                   MULT, ADD)
                    if s % 3 == 0:
                        # mode A: full chain on DVE
                        a3 = tmp_pool.tile([P, C], BF16, tag="a3")
                        nc.vector.scalar_tensor_tensor(a3[:], b0, w2, a2[:],
                                                       MULT, ADD)
                        nc.vector.scalar_tensor_tensor(o[:, s, :], b1, w3,
                                                       a3[:], MULT, ADD)
                    else:
                        # mode C: bottom corner pre-scaled on ACT, final
                        # add on the Pool ALU
                        a4 = tmp_pool.tile([P, C], BF16, tag="a4")
                        nc.scalar.mul(a4[:], b1, w3)
                        a3 = tmp_pool.tile([P, C], BF16, tag="a3")
                        nc.vector.scalar_tensor_tensor(a3[:], b0, w2, a4[:],
                                                       MULT, ADD)
                        nc.gpsimd.tensor_tensor(o[:, s, :], a2[:], a3[:], ADD)

                out_ap = bass.AP(out, ch * P * S * C, [(S * C, P), (1, S * C)])
                nc.sync.dma_start(out_ap, o[:])

            emit_gather(0)
            for ch in range(nch):
                if ch + 1 < nch:
                    emit_gather(ch + 1)
                emit_compute(ch)

    nc.compile()
    return nc


def kernel(image, boxes, box_ind):
    image = np.asarray(image, dtype=np.float32)
    boxes = np.asarray(boxes, dtype=np.float32)
    box_ind = np.asarray(box_ind)
    n_boxes = boxes.shape[0]

    # ring-balance: image j's boxes run on core j or (j-1)%8
    by_img = [np.where(box_ind == j)[0] for j in range(NCORES)]
    counts = [len(s) for s in by_img]
    L, z = _ring_balance(counts)
    m_pad = ((L + CH - 1) // CH) * CH
    nch = m_pad // CH
    dummy = np.array([[0.25, 0.25, 0.75, 0.75]], np.float32)

    # core k gets: kept boxes of image k (offset 0) then pushed boxes of
    # image k+1 (offset NPIX)
    sel = []
    offs = []
    for k in range(NCORES):
        j2 = (k + 1) % NCORES
        kept = by_img[k][z[k]:]
        pushed = by_img[j2][: z[j2]]
        sel.append(np.concatenate([kept, pushed]))
        offs.append(np.concatenate(
            [np.zeros(len(kept), np.int32),
             np.full(len(pushed), NPIX, np.int32)]))

    image16 = np.ascontiguousarray(
        image.transpose(0, 2, 3, 1)).astype(ml_dtypes.bfloat16)  # [B,H,W,C]
    patches = [_build_patches(image16[k]) for k in range(NCORES)]

    in_maps = []
    for k in range(NCORES):
        bk = boxes[sel[k]]
        ok_ = offs[k]
        if bk.shape[0] < m_pad:
            pad = m_pad - bk.shape[0]
            bk = np.concatenate([bk, np.repeat(dummy, pad, 0)], axis=0)
            ok_ = np.concatenate([ok_, np.zeros(pad, np.int32)])
        idx_w, w_all = _build_core_inputs(bk, ok_)
        in_maps.append({
            "img": np.concatenate([patches[k], patches[(k + 1) % NCORES]]),
            "idx": idx_w,
            "wts": w_all,
        })

    key = nch
    if key not in _cache:
        _cache[key] = _build_program(nch)
    nc = _cache[key]

    res = bass_utils.run_bass_kernel_spmd(nc, in_maps,
                                          core_ids=list(range(NCORES)))
    global LAST_EXEC_NS, LAST_RESULT
    LAST_EXEC_NS = res.exec_time_ns
    LAST_RESULT = res

    out = np.zeros((n_boxes, C, CROP, CROP), np.float32)
    for k in range(NCORES):
        ok = res.results[k]["out"].reshape(nch, P, S, C)
        ok = ok.transpose(0, 2, 1, 3).reshape(nch, QPAD, C)[:, : CH * PX]
        ok = ok.reshape(m_pad, PX, C)[: len(sel[k])].astype(np.float32)
        out[sel[k]] = ok.transpose(0, 2, 1).reshape(-1, C, CROP, CROP)
    return out
